# revision 9
# baseline (speedup 1.0000x reference)
"""Fused LayerNorm->MHA(multi-query)->LayerNorm kernel for TRN2, 8 cores SPMD.

Problem shapes (hardcoded):
  x:        [4, 2048, 512] f32
  attn_bias:[8, 2048, 2048] f32   (shared across batch)
  w_q:      [512, 512], w_kv: [512, 128], w_out: [512, 512]
  g_in, g_out: [512]
  out:      [4, 2048, 512] f32

Sharding: 8 cores = (batch b in 0..3) x (query-half ih in 0..1).
Each core computes the full pipeline for one batch and 1024 query rows.

v2 structure (vs v1 baseline):
  - LayerNorm rstd computed as exp(-0.5*ln(var+eps)) so the scalar engine
    needs only the natural_log_exp table set (no sqrt table switches), and
    the rstd computation is batched into two [128,16] activations.
  - LN scale/bias apply moved off the scalar engine onto DVE tensor_scalar
    (single-src 2x mode); the scalar engine runs (almost) only the 128
    softmax exp calls, which are its hard floor.
  - attn_bias (host-precomputed exp(bias)/16, fp16) streams on the GpSimd
    software-DGE queue in 2MiB chunks so it never competes with the small
    control DMAs on the Sync queue.
  - The bias multiply is one [128, 2048] fp16 tensor_tensor per j-tile
    (flat APs, separate output tile) to hit the DVE 2x 16-bit mode.
  - Softmax denominator: per head-pair, one [1,2048] row DMA -> scatter to
    [32,64] -> one reciprocal -> broadcast DMA -> one wide multiply, all off
    the critical path (the v1 per-unit DMA round-trips caused PE idle gaps
    long enough to re-throttle the PE clock).
"""

import sys

sys.path.insert(0, "/opt/trn_rl_repo")

import numpy as np
from contextlib import ExitStack

import concourse.bass as bass
import concourse.tile as tile
from concourse import bacc
from concourse import mybir
from concourse.masks import make_identity

B, N, DIM = 4, 2048, 512
HEADS, DH = 8, 64
INNER = HEADS * DH  # 512
EPS = 1e-5
SCALE = DH ** -0.5
NCORES = 8
IH = N // 2  # 1024 query rows per core
P = 128

NT = N // P      # 16 row tiles of x / j tiles
DT = DIM // P    # 4 d tiles
CT = INNER // P  # 4 c tiles (head pairs)
JT = N // P      # 16 j tiles
HP = HEADS // 2  # 4 head pairs
JPC = 4          # j tiles per bias DMA chunk (2 MiB chunks)

F32 = mybir.dt.float32
F16 = mybir.dt.float16

# Constant folded into exp(bias) on the host: keeps e = exp(s+b)/16 well
# inside fp16 range (max ~15k vs 65504). Cancels in the softmax ratio.
E_SCALE = 1.0 / 16.0

AF = mybir.ActivationFunctionType
ALU = mybir.AluOpType


def build_bass():
    nc = bacc.Bacc("TRN2")
    x_d = nc.dram_tensor("x", [N, DIM], F32, kind="ExternalInput")
    bias_d = nc.dram_tensor("biasT", [HP, JT, P, 2, IH], F16, kind="ExternalInput")
    wq_d = nc.dram_tensor("wq", [DIM, INNER], F16, kind="ExternalInput")
    wkv_d = nc.dram_tensor("wkv", [DIM, 2 * DH], F16, kind="ExternalInput")
    wout_d = nc.dram_tensor("wout", [INNER, DIM], F16, kind="ExternalInput")
    out_d = nc.dram_tensor("out", [IH, DIM], F32, kind="ExternalOutput")

    with tile.TileContext(nc) as tc:
        _body(tc, x_d, bias_d, wq_d, wkv_d, wout_d, out_d)
    nc.compile()
    return nc


def _flat(ap):
    """[P, a, b] AP -> [P, a*b] view."""
    return ap.rearrange("p a b -> p (a b)")


def _body(tc, x_d, bias_d, wq_d, wkv_d, wout_d, out_d):
    nc = tc.nc
    ctx = ExitStack()
    with ctx:
        persist = ctx.enter_context(tc.tile_pool(name="persist", bufs=1))

        # x loads first so LayerNorm starts immediately
        xload = ctx.enter_context(tc.tile_pool(name="xload", bufs=1))
        x_big = []
        for g in range(4):
            xg = xload.tile([P, 4, DIM], F32, name=f"x{g}")
            nc.sync.dma_start(
                out=xg,
                in_=x_d[g * 4 * P:(g + 1) * 4 * P, :]
                .rearrange("(t p) d -> p t d", p=P))
            x_big.append(xg)
        x_tiles = [x_big[nt // 4][:, nt % 4, :] for nt in range(NT)]

        # bias chunks: big streaming DMAs on the gpsimd (SWDGE) queue.
        biasp = ctx.enter_context(tc.tile_pool(name="biasp", bufs=2))

        def load_bias(hp, jp):
            ebt = biasp.tile([P, JPC, 2, IH], F16, name="eb")
            nc.sync.dma_start(
                out=ebt,
                in_=bias_d[hp, jp * JPC:(jp + 1) * JPC]
                .rearrange("t p h i -> p t h i"))
            return ebt

        eb_pending = {}
        eb_pending[(0, 0)] = load_bias(0, 0)

        identity = persist.tile([P, P], F16, name="identity")
        make_identity(nc, identity)
        eps_t = persist.tile([P, 1], F32, name="eps")
        nc.vector.memset(eps_t, EPS)

        # weights
        wq_sb = [persist.tile([P, INNER], F16, name=f"wq{d}") for d in range(DT)]
        wkv_sb = [persist.tile([P, 2 * DH], F16, name=f"wkv{d}") for d in range(DT)]
        wout_sb = [persist.tile([DH, DIM], F16, name=f"wout{h}")
                   for h in range(HEADS)]
        for d in range(DT):
            nc.sync.dma_start(out=wq_sb[d], in_=wq_d[d * P:(d + 1) * P, :])
            nc.sync.dma_start(out=wkv_sb[d], in_=wkv_d[d * P:(d + 1) * P, :])
        for h in range(HEADS):
            nc.sync.dma_start(out=wout_sb[h], in_=wout_d[h * DH:(h + 1) * DH, :])

        eb_pending[(0, 1)] = load_bias(0, 1)

        # persistent on-chip tensors
        xnT = persist.tile([P, DT, N], F16, name="xnT")   # [d-in-tile, dtile, n]
        kvT = persist.tile([P, N], F16, name="kvT")       # rows 0:64 v, 64:128 k
        kT2 = persist.tile([P, N], F16, name="kT2")       # k duplicated both halves
        vp = [persist.tile([P, DH + 1], F16, name=f"vp{j}") for j in range(JT)]
        qT = [persist.tile([P, IH], F16, name=f"qT{t}") for t in range(CT)]
        ao_un = [persist.tile([DH + 1, 4 * 512], F32, name=f"aoun{hp}")
                 for hp in range(HP)]
        aoT = [persist.tile([DH, 2 * IH], F16, name=f"aoT{hp}") for hp in range(HP)]
        denT8 = persist.tile([P, 64], F32, name="denT8")
        # batched LN stats
        vars16 = persist.tile([P, NT], F32, name="vars16")
        rstd16 = persist.tile([P, NT], F32, name="rstd16")
        negmr16 = persist.tile([P, NT], F32, name="negmr16")

        # ---- Phase 1a: LN statistics (all 16 tiles, batched rstd) ----
        with tc.tile_pool(name="ln", bufs=4) as ln:
            for nt in range(NT):
                stats = ln.tile([P, 6], F32, name="stats")
                nc.vector.bn_stats(out=stats, in_=x_tiles[nt])
                mv = ln.tile([P, 2], F32, name="mv")
                nc.vector.bn_aggr(out=mv, in_=stats)
                nc.vector.tensor_copy(out=vars16[:, nt:nt + 1], in_=mv[:, 1:2])
                nc.vector.tensor_copy(out=negmr16[:, nt:nt + 1], in_=mv[:, 0:1])
            # rstd = exp(-0.5 * ln(var + eps)); two wide activations
            lnv = ln.tile([P, NT], F32, name="lnv")
            nc.scalar.activation(out=lnv, in_=vars16, func=AF.Ln,
                                 bias=eps_t, scale=1.0)
            nc.scalar.activation(out=rstd16, in_=lnv, func=AF.Exp,
                                 bias=0.0, scale=-0.5)
            # negmr = -mean * rstd (negmr16 currently holds mean)
            nc.vector.scalar_tensor_tensor(
                out=negmr16, in0=negmr16, scalar=-1.0, in1=rstd16,
                op0=ALU.mult, op1=ALU.mult)

        # ---- Phase 1b + 2: xn, transposes, projections (per 512-chunk) ----
        with tc.tile_pool(name="xn", bufs=3) as xnp, \
             tc.tile_pool(name="lnps", bufs=2, space="PSUM") as lnps, \
             tc.tile_pool(name="projps", bufs=2, space="PSUM") as projps, \
             tc.tile_pool(name="vpps", bufs=2, space="PSUM") as vpps:
            for g in range(4):
                for t4 in range(4):
                    nt = g * 4 + t4
                    xn_t = xnp.tile([P, DIM], F16, name="xn_t")
                    nc.vector.tensor_scalar(
                        out=xn_t, in0=x_tiles[nt],
                        scalar1=rstd16[:, nt:nt + 1],
                        scalar2=negmr16[:, nt:nt + 1],
                        op0=ALU.mult, op1=ALU.add)
                    for d in range(DT):
                        ps = lnps.tile([P, P], F16, name="tps")
                        nc.tensor.transpose(ps, xn_t[:, d * P:(d + 1) * P],
                                            identity)
                        nc.vector.tensor_copy(
                            out=xnT[:, d, nt * P:(nt + 1) * P], in_=ps)
                lo, hi = g * 512, (g + 1) * 512
                # kv projection for this chunk
                ckv = projps.tile([P, 512], F32, name="ckv")
                for d in range(DT):
                    nc.tensor.matmul(ckv, wkv_sb[d], xnT[:, d, lo:hi],
                                     start=(d == 0), stop=(d == DT - 1))
                nc.vector.tensor_copy(out=kvT[:, lo:hi], in_=ckv)
                # k rows live at partitions 64:128; duplicate to both halves
                nc.vector.tensor_copy(out=kT2[DH:2 * DH, lo:hi],
                                      in_=kvT[DH:2 * DH, lo:hi])
                nc.sync.dma_start(out=kT2[0:DH, lo:hi], in_=kvT[DH:2 * DH, lo:hi])
                # v row tiles with ones column
                for j in range(g * 4, g * 4 + 4):
                    ps = vpps.tile([P, DH], F16, name="vps")
                    nc.tensor.transpose(ps, kvT[0:DH, j * P:(j + 1) * P],
                                        identity[0:DH, 0:DH])
                    nc.vector.tensor_copy(out=vp[j][:, 0:DH], in_=ps)
                    nc.vector.memset(vp[j][:, DH:DH + 1], 1.0)
                # q projection (only local-query chunks)
                if g < 2:
                    for t in range(CT):
                        qps = projps.tile([P, 512], F32, name="qps")
                        for d in range(DT):
                            nc.tensor.matmul(
                                qps, wq_sb[d][:, t * P:(t + 1) * P],
                                xnT[:, d, lo:hi],
                                start=(d == 0), stop=(d == DT - 1))
                        nc.vector.tensor_copy(out=qT[t][:, lo:hi], in_=qps)

        # ---- Phase 3: attention ----
        with tc.tile_pool(name="eraw", bufs=2) as erawp, \
             tc.tile_pool(name="emul", bufs=3) as emulp, \
             tc.tile_pool(name="bc", bufs=1) as bcp, \
             tc.tile_pool(name="dden", bufs=4, space="DRAM") as ddenp, \
             tc.tile_pool(name="qkps", bufs=2, space="PSUM") as qkps, \
             tc.tile_pool(name="avps", bufs=1, space="PSUM") as avps:
            for hp in range(HP):
                av = [[avps.tile([DH + 1, 512], F32, name=f"av{hh}_{ic}")
                       for ic in range(2)] for hh in range(2)]
                pend = None
                for jp in range(JT // JPC):
                    ebt = eb_pending.pop((hp, jp), None)
                    if ebt is None:
                        ebt = load_bias(hp, jp)
                    # prefetch next chunk
                    nhp, njp = hp, jp + 1
                    if njp == JT // JPC:
                        nhp, njp = hp + 1, 0
                    if nhp < HP and (nhp, njp) not in eb_pending:
                        eb_pending[(nhp, njp)] = load_bias(nhp, njp)
                    for jj in range(JPC):
                        j = jp * JPC + jj
                        # 1) QK matmuls for this unit (2 heads row-packed)
                        e_raw = erawp.tile([P, 2, IH], F16, name="e_raw")
                        for hh in range(2):
                            s = qkps.tile([P, IH], F32, name="s")
                            for ic in range(2):
                                nc.tensor.matmul(
                                    s[:, ic * 512:(ic + 1) * 512],
                                    kT2[hh * DH:(hh + 1) * DH,
                                        j * P:(j + 1) * P],
                                    qT[hp][hh * DH:(hh + 1) * DH,
                                           ic * 512:(ic + 1) * 512],
                                    start=True, stop=True,
                                    tile_position=(hh * DH, 0))
                            # 2) exp on ACT, one [128,1024] call per head
                            nc.scalar.activation(
                                out=e_raw[:, hh, :], in_=s, func=AF.Exp)
                        # 3) previous unit's A@V (PE never stalls on DVE here)
                        if pend is not None:
                            pj, pe_t = pend
                            for hh in range(2):
                                for ic in range(2):
                                    nc.tensor.matmul(
                                        av[hh][ic], vp[pj],
                                        pe_t[:, hh, ic * 512:(ic + 1) * 512],
                                        start=(pj == 0), stop=(pj == JT - 1))
                            pend = None
                        # 4) one wide fp16 bias multiply on DVE
                        e_t = emulp.tile([P, 2, IH], F16, name="e_t")
                        nc.vector.tensor_tensor(
                            _flat(e_t[:]), _flat(e_raw[:]),
                            _flat(ebt[:, jj]), ALU.mult)
                        pend = (j, e_t)
                # drain the last pipelined unit
                if pend is not None:
                    pj, pe_t = pend
                    for hh in range(2):
                        for ic in range(2):
                            nc.tensor.matmul(
                                av[hh][ic], vp[pj],
                                pe_t[:, hh, ic * 512:(ic + 1) * 512],
                                start=(pj == 0), stop=(pj == JT - 1))
                    pend = None
                # evacuate PSUM (unnormalized attnout + den row together)
                for hh in range(2):
                    for ic in range(2):
                        u = hh * 2 + ic
                        nc.vector.tensor_copy(
                            out=ao_un[hp][:, u * 512:(u + 1) * 512],
                            in_=av[hh][ic])
                # denominator: row 64 of ao_un -> scatter [32,64] -> recip
                # -> broadcast back over 64 partitions -> one wide multiply
                dd = ddenp.tile([1, 4 * 512], F32, name="dd")
                nc.sync.dma_start(out=dd, in_=ao_un[hp][DH:DH + 1, :])
                dd_ap = dd[:] if not isinstance(dd, bass.AP) else dd
                nc.sync.dma_start(
                    out=denT8[hp * 32:(hp + 1) * 32, :],
                    in_=bass.AP(tensor=dd_ap.tensor, offset=dd_ap.offset,
                                ap=[[64, 32], [1, 64]]))
                nc.vector.reciprocal(out=denT8[hp * 32:(hp + 1) * 32, :],
                                     in_=denT8[hp * 32:(hp + 1) * 32, :])
                dd2 = ddenp.tile([1, 4 * 512], F32, name="dd2")
                dd2_ap = dd2[:] if not isinstance(dd2, bass.AP) else dd2
                nc.sync.dma_start(
                    out=bass.AP(tensor=dd2_ap.tensor, offset=dd2_ap.offset,
                                ap=[[64, 32], [1, 64]]),
                    in_=denT8[hp * 32:(hp + 1) * 32, :])
                bc = bcp.tile([DH, 4 * 512], F32, name="bc")
                nc.sync.dma_start(
                    out=bc,
                    in_=bass.AP(tensor=dd2_ap.tensor, offset=dd2_ap.offset,
                                ap=[[0, DH], [1, 4 * 512]]))
                nc.vector.tensor_tensor(
                    aoT[hp], ao_un[hp][0:DH, :], bc, ALU.mult)

        # ---- Phase 4: output projection + LayerNorm ----
        with tc.tile_pool(name="fin", bufs=3) as fin, \
             tc.tile_pool(name="finps", bufs=2, space="PSUM") as finps:
            vars8 = fin.tile([P, 8], F32, name="vars8")
            negmr8 = fin.tile([P, 8], F32, name="negmr8")
            rstd8 = fin.tile([P, 8], F32, name="rstd8")
            o_copies = []
            for it in range(IH // P):
                o_ps = finps.tile([P, DIM], F32, name="o_ps")
                for h in range(HEADS):
                    t, hh = divmod(h, 2)
                    nc.tensor.matmul(
                        o_ps,
                        aoT[t][:, hh * IH + it * P:hh * IH + (it + 1) * P],
                        wout_sb[h],
                        start=(h == 0), stop=(h == HEADS - 1))
                o_sb = persist.tile([P, DIM], F32, name=f"o_sb{it}")
                nc.vector.tensor_copy(out=o_sb, in_=o_ps)
                o_copies.append(o_sb)
                stats = fin.tile([P, 6], F32, name="stats")
                nc.vector.bn_stats(out=stats, in_=o_sb)
                mv = fin.tile([P, 2], F32, name="mv")
                nc.vector.bn_aggr(out=mv, in_=stats)
                nc.vector.tensor_copy(out=vars8[:, it:it + 1], in_=mv[:, 1:2])
                nc.vector.tensor_copy(out=negmr8[:, it:it + 1], in_=mv[:, 0:1])
            lnv8 = fin.tile([P, 8], F32, name="lnv8")
            nc.scalar.activation(out=lnv8, in_=vars8, func=AF.Ln,
                                 bias=eps_t, scale=1.0)
            nc.scalar.activation(out=rstd8, in_=lnv8, func=AF.Exp,
                                 bias=0.0, scale=-0.5)
            nc.vector.scalar_tensor_tensor(
                out=negmr8, in0=negmr8, scalar=-1.0, in1=rstd8,
                op0=ALU.mult, op1=ALU.mult)
            for it in range(IH // P):
                o_out = fin.tile([P, DIM], F32, name="o_out")
                nc.vector.tensor_scalar(
                    out=o_out, in0=o_copies[it],
                    scalar1=rstd8[:, it:it + 1],
                    scalar2=negmr8[:, it:it + 1],
                    op0=ALU.mult, op1=ALU.add)
                nc.sync.dma_start(out=out_d[it * P:(it + 1) * P, :], in_=o_out)


_NC_CACHE = None


def _get_nc():
    global _NC_CACHE
    if _NC_CACHE is None:
        _NC_CACHE = build_bass()
    return _NC_CACHE


def make_in_maps(x, attn_bias, w_q, w_kv, w_out, g_in, g_out):
    x = np.asarray(x, np.float32)
    attn_bias = np.asarray(attn_bias, np.float32)
    g_in = np.asarray(g_in, np.float32)
    wq_eff = np.ascontiguousarray(
        ((g_in[:, None] * np.asarray(w_q, np.float32)) * SCALE).astype(np.float16))
    wkv = g_in[:, None] * np.asarray(w_kv, np.float32)
    # reorder kv projection columns to [v, k]
    wkv_eff = np.ascontiguousarray(
        np.concatenate([wkv[:, DH:], wkv[:, :DH]], axis=1).astype(np.float16))
    w_out = np.ascontiguousarray(
        np.asarray(w_out, np.float32).astype(np.float16))
    # exp(bias)/16, transposed to [h, j, i]
    biasT = np.transpose(attn_bias, (0, 2, 1))
    in_maps = []
    for c in range(NCORES):
        b, ih = divmod(c, 2)
        lo, hi = ih * IH, (ih + 1) * IH
        # local query rows first; k/v row order is irrelevant to the math
        # as long as the bias j-rows are permuted identically
        xp = np.concatenate([x[b, lo:hi], x[b, :lo], x[b, hi:]], axis=0)
        bj = np.concatenate(
            [biasT[:, lo:hi, lo:hi], biasT[:, :lo, lo:hi], biasT[:, hi:, lo:hi]],
            axis=1)
        ebj = (np.exp(bj) * E_SCALE).astype(np.float16)
        # [h=8, j=2048, i=1024] -> [hp, jt, p, hh, i]
        eb6 = np.ascontiguousarray(
            ebj.reshape(HP, 2, JT, P, IH).transpose(0, 2, 3, 1, 4))
        in_maps.append({
            "x": np.ascontiguousarray(xp),
            "biasT": eb6,
            "wq": wq_eff, "wkv": wkv_eff, "wout": w_out,
        })
    return in_maps


def assemble(results):
    out = np.empty((B, N, DIM), np.float32)
    for c in range(NCORES):
        b, ih = divmod(c, 2)
        out[b, ih * IH:(ih + 1) * IH, :] = results[c]["out"]
    return out


def kernel(x, attn_bias, w_q, w_kv, w_out, g_in, g_out):
    from concourse.bass_utils import run_bass_kernel_spmd

    in_maps = make_in_maps(x, attn_bias, w_q, w_kv, w_out, g_in, g_out)
    nc = _get_nc()
    res = run_bass_kernel_spmd(nc, in_maps, list(range(NCORES))).results
    return assemble(res) * np.asarray(g_out, np.float32)[None, None, :]


# revision 15
# speedup vs baseline: 1.0627x; 1.0627x over previous
"""Fused LayerNorm->MHA(multi-query)->LayerNorm kernel for TRN2, 8 cores SPMD.

Problem shapes (hardcoded):
  x:        [4, 2048, 512] f32
  attn_bias:[8, 2048, 2048] f32   (shared across batch)
  w_q:      [512, 512], w_kv: [512, 128], w_out: [512, 512]
  g_in, g_out: [512]
  out:      [4, 2048, 512] f32

Sharding: 8 cores = (batch b in 0..3) x (query-half ih in 0..2).
Each core computes the full pipeline for one batch and 1024 query rows.

v3 structure:
  - Attention inner loop is software-pipelined at depth 2 (A@V for unit j-2
    runs between the QK matmuls of unit j), so the PE never sits in the
    ACT->DVE->PE dependency cycle and the exp stream on the scalar engine
    runs back to back.
  - A@V stationary operands are padded to 128 columns with the softmax-ones
    column placed so head hh=0 lands on PSUM partitions 0..64 and hh=1 on
    63..127.  The packed [128, 1024] attention output per head pair feeds an
    output projection of 4 full-contraction matmuls per row tile (vs 8
    half-contraction ones).
  - e tensors are bf16 (chasing the DVE 2x 16-bit tensor_tensor mode).
  - LayerNorm rstd = 1/sqrt(var+eps) via one batched [128,4] Sqrt + DVE
    reciprocal per 4-tile chunk; the scale/bias apply runs on DVE
    tensor_scalar.  A dummy sqrt after the last exp preloads the sqrt table
    so the phase-4 table switch is off the critical tail.
  - Softmax denominator handled per head pair with one DMA round trip
    (scatter -> [32,64] reciprocal -> broadcast) feeding two wide multiplies.
"""

import sys

sys.path.insert(0, "/opt/trn_rl_repo")

import numpy as np
from contextlib import ExitStack

import concourse.bass as bass
import concourse.tile as tile
from concourse import bacc
from concourse import mybir
from concourse.masks import make_identity

B, N, DIM = 4, 2048, 512
HEADS, DH = 8, 64
INNER = HEADS * DH  # 512
EPS = 1e-5
SCALE = DH ** -0.5
NCORES = 8
IH = N // 2  # 1024 query rows per core
P = 128

NT = N // P      # 16 row tiles of x / j tiles
DT = DIM // P    # 4 d tiles
CT = INNER // P  # 4 c tiles
JT = N // P      # 16 j tiles
HP = HEADS // 2  # 4 head pairs
JPC = 4          # j tiles per bias DMA chunk (2 MiB chunks)

F32 = mybir.dt.float32
F16 = mybir.dt.float16
BF16 = mybir.dt.bfloat16

# Constant folded into exp(bias) on the host: keeps e = exp(s+b)/16 well
# inside 16-bit float range. Cancels in the softmax ratio.
E_SCALE = 1.0 / 16.0

AF = mybir.ActivationFunctionType
ALU = mybir.AluOpType


def build_bass():
    nc = bacc.Bacc("TRN2")
    x_d = nc.dram_tensor("x", [N, DIM], F32, kind="ExternalInput")
    bias_d = nc.dram_tensor("biasT", [HP, JT, P, 2, IH], BF16, kind="ExternalInput")
    wq_d = nc.dram_tensor("wq", [DIM, INNER], F16, kind="ExternalInput")
    wkv_d = nc.dram_tensor("wkv", [DIM, 2 * DH], F16, kind="ExternalInput")
    wout_d = nc.dram_tensor("wout", [INNER, DIM], F16, kind="ExternalInput")
    out_d = nc.dram_tensor("out", [IH, DIM], F32, kind="ExternalOutput")

    with tile.TileContext(nc) as tc:
        _body(tc, x_d, bias_d, wq_d, wkv_d, wout_d, out_d)
    nc.compile()
    return nc


def _body(tc, x_d, bias_d, wq_d, wkv_d, wout_d, out_d):
    nc = tc.nc
    ctx = ExitStack()
    with ctx:
        persist = ctx.enter_context(tc.tile_pool(name="persist", bufs=1))

        # x loads first so LayerNorm starts immediately
        xload = ctx.enter_context(tc.tile_pool(name="xload", bufs=1))
        x_big = []
        for g in range(4):
            xg = xload.tile([P, 4, DIM], F32, name=f"x{g}")
            nc.sync.dma_start(
                out=xg,
                in_=x_d[g * 4 * P:(g + 1) * 4 * P, :]
                .rearrange("(t p) d -> p t d", p=P))
            x_big.append(xg)
        x_tiles = [x_big[nt // 4][:, nt % 4, :] for nt in range(NT)]

        biasp = ctx.enter_context(tc.tile_pool(name="biasp", bufs=2))

        def load_bias(hp, jp):
            ebt = biasp.tile([P, JPC, 2, IH], BF16, name="eb")
            nc.sync.dma_start(
                out=ebt,
                in_=bias_d[hp, jp * JPC:(jp + 1) * JPC]
                .rearrange("t p h i -> p t h i"))
            return ebt

        eb_pending = {(0, 0): load_bias(0, 0)}

        identity = persist.tile([P, P], F16, name="identity")
        make_identity(nc, identity)
        eps_t = persist.tile([P, 1], F32, name="eps")
        nc.vector.memset(eps_t, EPS)

        # weights
        wq_sb = [persist.tile([P, INNER], F16, name=f"wq{d}") for d in range(DT)]
        wkv_sb = [persist.tile([P, 2 * DH], F16, name=f"wkv{d}") for d in range(DT)]
        wout_sb = [persist.tile([P, DIM], F16, name=f"wout{hp}")
                   for hp in range(HP)]
        for d in range(DT):
            nc.sync.dma_start(out=wq_sb[d], in_=wq_d[d * P:(d + 1) * P, :])
            nc.sync.dma_start(out=wkv_sb[d], in_=wkv_d[d * P:(d + 1) * P, :])
        for hp in range(HP):
            nc.sync.dma_start(out=wout_sb[hp], in_=wout_d[hp * P:(hp + 1) * P, :])

        eb_pending[(0, 1)] = load_bias(0, 1)

        # persistent on-chip tensors
        xnT = persist.tile([P, DT, N], F16, name="xnT")   # [d-in-tile, dtile, n]
        kvT = persist.tile([P, N], F16, name="kvT")       # rows 0:64 v, 64:128 k
        kT2 = persist.tile([P, N], F16, name="kT2")       # k duplicated both halves
        # padded A@V stationaries: vpA = [v | ones | 0], vpB = [0 | ones | v]
        vpA = [persist.tile([P, P], BF16, name=f"vpA{j}") for j in range(JT)]
        vpB = [persist.tile([P, P], BF16, name=f"vpB{j}") for j in range(JT)]
        qT = [persist.tile([P, IH], F16, name=f"qT{t}") for t in range(CT)]
        aoA = [persist.tile([DH + 1, 2 * 512], F32, name=f"aoA{hp}")
               for hp in range(HP)]
        aoB = [persist.tile([P, 2 * 512], F32, name=f"aoB{hp}")
               for hp in range(HP)]
        aoT = [persist.tile([P, 2 * 512], F16, name=f"aoT{hp}")
               for hp in range(HP)]
        denT8 = persist.tile([P, 64], F32, name="denT8")

        # ---- Phase 1+2: LN, transposes, projections, per 512-row chunk ----
        with tc.tile_pool(name="ln", bufs=3) as ln, \
             tc.tile_pool(name="lnps", bufs=2, space="PSUM") as lnps, \
             tc.tile_pool(name="projps", bufs=2, space="PSUM") as projps, \
             tc.tile_pool(name="vpps", bufs=2, space="PSUM") as vpps:
            for g in range(4):
                mean4 = ln.tile([P, 4], F32, name="mean4")
                var4 = ln.tile([P, 4], F32, name="var4")
                rstd4 = ln.tile([P, 4], F32, name="rstd4")
                for t4 in range(4):
                    nt = g * 4 + t4
                    stats = ln.tile([P, 6], F32, name="stats")
                    nc.vector.bn_stats(out=stats, in_=x_tiles[nt])
                    mv = ln.tile([P, 2], F32, name="mv")
                    nc.vector.bn_aggr(out=mv, in_=stats)
                    nc.vector.tensor_copy(out=var4[:, t4:t4 + 1], in_=mv[:, 1:2])
                    nc.vector.tensor_copy(out=mean4[:, t4:t4 + 1], in_=mv[:, 0:1])
                nc.scalar.activation(out=rstd4, in_=var4, func=AF.Sqrt,
                                     bias=eps_t, scale=1.0)
                nc.vector.reciprocal(out=rstd4, in_=rstd4)
                # negmr = -mean * rstd
                nc.vector.scalar_tensor_tensor(
                    out=mean4, in0=mean4, scalar=-1.0, in1=rstd4,
                    op0=ALU.mult, op1=ALU.mult)
                for t4 in range(4):
                    nt = g * 4 + t4
                    xn_t = ln.tile([P, DIM], F16, name="xn_t")
                    nc.vector.tensor_scalar(
                        out=xn_t, in0=x_tiles[nt],
                        scalar1=rstd4[:, t4:t4 + 1],
                        scalar2=mean4[:, t4:t4 + 1],
                        op0=ALU.mult, op1=ALU.add)
                    for d in range(DT):
                        ps = lnps.tile([P, P], F16, name="tps")
                        nc.tensor.transpose(ps, xn_t[:, d * P:(d + 1) * P],
                                            identity)
                        nc.vector.tensor_copy(
                            out=xnT[:, d, nt * P:(nt + 1) * P], in_=ps)
                lo, hi = g * 512, (g + 1) * 512
                # kv projection for this chunk
                ckv = projps.tile([P, 512], F32, name="ckv")
                for d in range(DT):
                    nc.tensor.matmul(ckv, wkv_sb[d], xnT[:, d, lo:hi],
                                     start=(d == 0), stop=(d == DT - 1))
                nc.vector.tensor_copy(out=kvT[:, lo:hi], in_=ckv)
                # k rows live at partitions 64:128; duplicate to both halves
                nc.vector.tensor_copy(out=kT2[DH:2 * DH, lo:hi],
                                      in_=kvT[DH:2 * DH, lo:hi])
                nc.sync.dma_start(out=kT2[0:DH, lo:hi], in_=kvT[DH:2 * DH, lo:hi])
                # padded v stationaries
                for j in range(g * 4, g * 4 + 4):
                    ps = vpps.tile([P, DH], F16, name="vps")
                    nc.tensor.transpose(ps, kvT[0:DH, j * P:(j + 1) * P],
                                        identity[0:DH, 0:DH])
                    nc.vector.memset(vpA[j][:, DH + 1:], 0.0)
                    nc.vector.tensor_copy(out=vpA[j][:, 0:DH], in_=ps)
                    nc.vector.memset(vpA[j][:, DH:DH + 1], 1.0)
                    nc.vector.memset(vpB[j][:, 0:DH - 1], 0.0)
                    nc.vector.tensor_copy(out=vpB[j][:, DH:], in_=ps)
                    nc.vector.memset(vpB[j][:, DH - 1:DH], 1.0)
                # q projection (only local-query chunks)
                if g < 2:
                    for t in range(CT):
                        qps = projps.tile([P, 512], F32, name="qps")
                        for d in range(DT):
                            nc.tensor.matmul(
                                qps, wq_sb[d][:, t * P:(t + 1) * P],
                                xnT[:, d, lo:hi],
                                start=(d == 0), stop=(d == DT - 1))
                        nc.vector.tensor_copy(out=qT[t][:, lo:hi], in_=qps)

        # ---- Phase 3: attention ----
        with tc.tile_pool(name="emul", bufs=4) as emulp, \
             tc.tile_pool(name="eraw", bufs=3) as erawp, \
             tc.tile_pool(name="bc", bufs=1) as bcp, \
             tc.tile_pool(name="dden", bufs=4, space="DRAM") as ddenp, \
             tc.tile_pool(name="qkps", bufs=2, space="PSUM") as qkps, \
             tc.tile_pool(name="avps", bufs=1, space="PSUM") as avps:
            for hp in range(HP):
                # av banks: ic -> [A (hh0: av@0:64, den@64), B (hh1: den@63,
                # av@64:128)]
                avb = [[avps.tile([P, 512], F32, name=f"av{ab}_{ic}")
                        for ab in range(2)] for ic in range(2)]
                pend = []
                for jp in range(JT // JPC):
                    ebt = eb_pending.pop((hp, jp), None)
                    if ebt is None:
                        ebt = load_bias(hp, jp)
                    nhp, njp = (hp, jp + 1) if jp + 1 < JT // JPC else (hp + 1, 0)
                    if nhp < HP and (nhp, njp) not in eb_pending:
                        eb_pending[(nhp, njp)] = load_bias(nhp, njp)
                    for jj in range(JPC):
                        j = jp * JPC + jj
                        # QK (2 heads row-packed), exp, bias-mult per head
                        e_t = emulp.tile([P, 2, IH], BF16, name="e_t")
                        for hh in range(2):
                            s = qkps.tile([P, IH], F32, name="s")
                            for ic in range(2):
                                nc.tensor.matmul(
                                    s[:, ic * 512:(ic + 1) * 512],
                                    kT2[hh * DH:(hh + 1) * DH,
                                        j * P:(j + 1) * P],
                                    qT[hp][hh * DH:(hh + 1) * DH,
                                           ic * 512:(ic + 1) * 512],
                                    start=True, stop=True,
                                    tile_position=(hh * DH, 0))
                            e_raw = erawp.tile([P, IH], BF16, name="e_raw")
                            nc.scalar.activation(
                                out=e_raw, in_=s, func=AF.Exp)
                            nc.vector.tensor_tensor(
                                e_t[:, hh, :], e_raw, ebt[:, jj, hh, :],
                                ALU.mult)
                        # A@V for the unit two back (depth-2 pipeline)
                        if len(pend) == 2:
                            pj, pe_t = pend.pop(0)
                            for ic in range(2):
                                for ab in range(2):
                                    nc.tensor.matmul(
                                        avb[ic][ab],
                                        (vpA if ab == 0 else vpB)[pj],
                                        pe_t[:, ab, ic * 512:(ic + 1) * 512],
                                        start=(pj == 0), stop=(pj == JT - 1))
                        pend.append((j, e_t))
                # drain
                for pj, pe_t in pend:
                    for ic in range(2):
                        for ab in range(2):
                            nc.tensor.matmul(
                                avb[ic][ab],
                                (vpA if ab == 0 else vpB)[pj],
                                pe_t[:, ab, ic * 512:(ic + 1) * 512],
                                start=(pj == 0), stop=(pj == JT - 1))
                pend = []
                # evacuate PSUM promptly (av + den rows together)
                for ic in range(2):
                    nc.vector.tensor_copy(
                        out=aoA[hp][:, ic * 512:(ic + 1) * 512],
                        in_=avb[ic][0][0:DH + 1, :])
                    nc.vector.tensor_copy(
                        out=aoB[hp][32:DH, ic * 512:(ic + 1) * 512],
                        in_=avb[ic][1][32:DH, :])
                    nc.vector.tensor_copy(
                        out=aoB[hp][DH:, ic * 512:(ic + 1) * 512],
                        in_=avb[ic][1][DH:, :])
                # denominator chain (off critical path)
                dd = ddenp.tile([1, 2048], F32, name="dd")
                dd_ap = dd[:] if not isinstance(dd, bass.AP) else dd
                nc.sync.dma_start(out=bass.AP(tensor=dd_ap.tensor,
                                              offset=dd_ap.offset,
                                              ap=[[1, 1024]]),
                                  in_=aoA[hp][DH:DH + 1, :])
                nc.sync.dma_start(out=bass.AP(tensor=dd_ap.tensor,
                                              offset=dd_ap.offset + 1024,
                                              ap=[[1, 1024]]),
                                  in_=aoB[hp][DH - 1:DH, :])
                nc.sync.dma_start(
                    out=denT8[hp * 32:(hp + 1) * 32, :],
                    in_=bass.AP(tensor=dd_ap.tensor, offset=dd_ap.offset,
                                ap=[[64, 32], [1, 64]]))
                nc.vector.reciprocal(out=denT8[hp * 32:(hp + 1) * 32, :],
                                     in_=denT8[hp * 32:(hp + 1) * 32, :])
                dd2 = ddenp.tile([1, 2048], F32, name="dd2")
                dd2_ap = dd2[:] if not isinstance(dd2, bass.AP) else dd2
                nc.sync.dma_start(
                    out=bass.AP(tensor=dd2_ap.tensor, offset=dd2_ap.offset,
                                ap=[[64, 32], [1, 64]]),
                    in_=denT8[hp * 32:(hp + 1) * 32, :])
                bc = bcp.tile([P, 2 * 512], F32, name="bc")
                nc.sync.dma_start(
                    out=bc[0:DH, :],
                    in_=bass.AP(tensor=dd2_ap.tensor, offset=dd2_ap.offset,
                                ap=[[0, DH], [1, 1024]]))
                nc.sync.dma_start(
                    out=bc[DH:, :],
                    in_=bass.AP(tensor=dd2_ap.tensor,
                                offset=dd2_ap.offset + 1024,
                                ap=[[0, DH], [1, 1024]]))
                nc.vector.tensor_tensor(
                    aoT[hp][0:DH, :], aoA[hp][0:DH, :], bc[0:DH, :], ALU.mult)
                nc.vector.tensor_tensor(
                    aoT[hp][DH:, :], aoB[hp][DH:, :], bc[DH:, :], ALU.mult)
            # preload the sqrt table while the phase-3/4 tail drains
            dummy = bcp.tile([1, 1], F32, name="sqdummy")
            nc.scalar.activation(out=dummy, in_=eps_t[0:1, :], func=AF.Sqrt)

        # ---- Phase 4: output projection + LayerNorm ----
        with tc.tile_pool(name="fin", bufs=3) as fin, \
             tc.tile_pool(name="finps", bufs=2, space="PSUM") as finps:
            for it in range(IH // P):
                o_ps = finps.tile([P, DIM], F32, name="o_ps")
                for hp in range(HP):
                    nc.tensor.matmul(
                        o_ps, aoT[hp][:, it * P:(it + 1) * P], wout_sb[hp],
                        start=(hp == 0), stop=(hp == HP - 1))
                stats = fin.tile([P, 6], F32, name="stats")
                nc.vector.bn_stats(out=stats, in_=o_ps)
                mv = fin.tile([P, 2], F32, name="mv")
                nc.vector.bn_aggr(out=mv, in_=stats)
                rstd = fin.tile([P, 1], F32, name="rstd")
                nc.scalar.activation(out=rstd, in_=mv[:, 1:2], func=AF.Sqrt,
                                     bias=eps_t, scale=1.0)
                nc.vector.reciprocal(out=rstd, in_=rstd)
                negmr = fin.tile([P, 1], F32, name="negmr")
                nc.vector.scalar_tensor_tensor(
                    out=negmr, in0=mv[:, 0:1], scalar=-1.0, in1=rstd,
                    op0=ALU.mult, op1=ALU.mult)
                o_sb = fin.tile([P, DIM], F32, name="o_sb")
                nc.vector.tensor_scalar(
                    out=o_sb, in0=o_ps,
                    scalar1=rstd, scalar2=negmr,
                    op0=ALU.mult, op1=ALU.add)
                nc.sync.dma_start(out=out_d[it * P:(it + 1) * P, :], in_=o_sb)


_NC_CACHE = None


def _get_nc():
    global _NC_CACHE
    if _NC_CACHE is None:
        _NC_CACHE = build_bass()
    return _NC_CACHE


def make_in_maps(x, attn_bias, w_q, w_kv, w_out, g_in, g_out):
    x = np.asarray(x, np.float32)
    attn_bias = np.asarray(attn_bias, np.float32)
    g_in = np.asarray(g_in, np.float32)
    wq_eff = np.ascontiguousarray(
        ((g_in[:, None] * np.asarray(w_q, np.float32)) * SCALE).astype(np.float16))
    wkv = g_in[:, None] * np.asarray(w_kv, np.float32)
    # reorder kv projection columns to [v, k]
    wkv_eff = np.ascontiguousarray(
        np.concatenate([wkv[:, DH:], wkv[:, :DH]], axis=1).astype(np.float16))
    w_out = np.ascontiguousarray(
        np.asarray(w_out, np.float32).astype(np.float16))
    np_bf16 = mybir.dt.np(BF16)
    biasT = np.transpose(attn_bias, (0, 2, 1))
    in_maps = []
    for c in range(NCORES):
        b, ih = divmod(c, 2)
        lo, hi = ih * IH, (ih + 1) * IH
        # local query rows first; k/v row order is irrelevant to the math
        # as long as the bias j-rows are permuted identically
        xp = np.concatenate([x[b, lo:hi], x[b, :lo], x[b, hi:]], axis=0)
        bj = np.concatenate(
            [biasT[:, lo:hi, lo:hi], biasT[:, :lo, lo:hi], biasT[:, hi:, lo:hi]],
            axis=1)
        ebj = (np.exp(bj) * E_SCALE).astype(np_bf16)
        # [h=8, j=2048, i=1024] -> [hp, jt, p, hh, i]
        eb6 = np.ascontiguousarray(
            ebj.reshape(HP, 2, JT, P, IH).transpose(0, 2, 3, 1, 4))
        in_maps.append({
            "x": np.ascontiguousarray(xp),
            "biasT": eb6,
            "wq": wq_eff, "wkv": wkv_eff, "wout": w_out,
        })
    return in_maps


def assemble(results):
    out = np.empty((B, N, DIM), np.float32)
    for c in range(NCORES):
        b, ih = divmod(c, 2)
        out[b, ih * IH:(ih + 1) * IH, :] = results[c]["out"]
    return out


def kernel(x, attn_bias, w_q, w_kv, w_out, g_in, g_out):
    from concourse.bass_utils import run_bass_kernel_spmd

    in_maps = make_in_maps(x, attn_bias, w_q, w_kv, w_out, g_in, g_out)
    nc = _get_nc()
    res = run_bass_kernel_spmd(nc, in_maps, list(range(NCORES))).results
    return assemble(res) * np.asarray(g_out, np.float32)[None, None, :]
